# revision 10
# baseline (speedup 1.0000x reference)
"""Trainium2 Bass kernel for a 2-layer GCN (GraphSAINTNet) on 8 NeuronCores.

Strategy (graph/data parallel, dst-sharded):
  * Nodes are padded to NPAD = 8*98*128 and partitioned into 8 contiguous
    shards (one per core), 98 blocks of 128 dst-nodes per core.
  * Edges (incl. self-loops) are sorted by dst on the host; each core gets
    the edges whose dst lands in its shard, laid out as fixed-size tiles of
    128 edges, TPB tiles per dst-block (padded with null edges).
  * Math restructure: out = Anorm @ ((relu(BN(Anorm @ X @ W1))) @ W2) + b2
    with Anorm = D^-1/2 (A+I) D^-1/2.  Aggregation commutes with the right
    matmuls, so layer 1 aggregates width-128 X and layer 2 aggregates
    width-64 (Y1 @ W2).  b1 vanishes inside BN (shift invariance).
  * Per 128-edge tile on device: indirect-DMA gather of source rows,
    per-edge norm scaling on the Scalar engine, one-hot selection matrix
    via is_equal on the Vector engine, and a PE matmul accumulating the
    dst-block result in PSUM.
  * BN stats via free-axis reduction in feature-major layout + AllReduce;
    layer-2 input exchanged with an AllGather.
"""

import numpy as np

import concourse.bacc as bacc
import concourse.bass as bass
import concourse.mybir as mybir
import concourse.tile as tile
from concourse import bass_utils
from concourse.masks import make_identity

F32 = mybir.dt.float32
I32 = mybir.dt.int32
AF = mybir.ActivationFunctionType
ALU = mybir.AluOpType

CORES = 8
BLK = 128
EPS = 1e-5


def _host_prep(edge_index, n_nodes, nb_pc, tpb_round=4):
    """Sort/shard/pad edges on the host. Returns per-(block,tile) layout arrays.

    Output arrays have shape [128, NBLOCKS*TPB]; column b*TPB+t holds edge-tile
    t of dst-block b (partition p = edge slot p).
    """
    nblocks = nb_pc * CORES
    src = np.asarray(edge_index[0], dtype=np.int64)
    dst = np.asarray(edge_index[1], dtype=np.int64)
    loop = np.arange(n_nodes, dtype=np.int64)
    s = np.concatenate([src, loop])
    d = np.concatenate([dst, loop])
    deg = np.bincount(d, minlength=n_nodes).astype(np.float64)
    dis = 1.0 / np.sqrt(deg)  # deg >= 1 due to self-loops
    norm = (dis[s] * dis[d]).astype(np.float32)

    order = np.argsort(d, kind="stable")
    ss, ds, ns = s[order], d[order], norm[order]

    blk_of_edge = ds // BLK
    counts = np.bincount(blk_of_edge, minlength=nblocks)
    tpb = int(-(-counts.max() // BLK))  # ceil
    tpb = -(-tpb // tpb_round) * tpb_round  # round up to multiple of G

    starts = np.zeros(nblocks + 1, dtype=np.int64)
    np.cumsum(counts, out=starts[1:])
    cc = np.arange(len(ss), dtype=np.int64) - starts[blk_of_edge]
    p = cc % BLK
    col = blk_of_edge * tpb + cc // BLK

    idx = np.zeros((BLK, nblocks * tpb), dtype=np.int32)
    lj = np.full((BLK, nblocks * tpb), 999.0, dtype=np.float32)
    nm = np.zeros((BLK, nblocks * tpb), dtype=np.float32)
    idx[p, col] = ss.astype(np.int32)
    lj[p, col] = (ds - blk_of_edge * BLK).astype(np.float32)
    nm[p, col] = ns
    return idx, lj, nm, tpb


def build_kernel(n_nodes, n_in, n_hid, n_out, nb_pc, tpb, gather_g=1, gather_g2=1, dbg=False, eq_batch=4):
    """Build the SPMD Bass program (same NEFF for all 8 cores)."""
    assert n_in == 128 and n_hid == 256 and n_out <= 128
    assert tpb % gather_g == 0
    nsh = nb_pc * BLK           # nodes per shard
    npad = nsh * CORES
    ncols = nb_pc * tpb         # edge-tile columns per core
    inv_n = 1.0 / float(n_nodes)

    nc = bacc.Bacc(
        "TRN2",
        target_bir_lowering=False,
        debug=False,
        enable_asserts=False,
        num_devices=CORES,
    )

    x_t = nc.dram_tensor("x", [n_nodes, n_in], F32, kind="ExternalInput")
    idx_t = nc.dram_tensor("eidx", [BLK, ncols], I32, kind="ExternalInput")
    lj_t = nc.dram_tensor("elj", [BLK, ncols], F32, kind="ExternalInput")
    nm_t = nc.dram_tensor("enm", [BLK, ncols], F32, kind="ExternalInput")
    w1_t = nc.dram_tensor("w1", [n_in, n_hid], F32, kind="ExternalInput")
    w2_t = nc.dram_tensor("w2", [n_hid, n_out], F32, kind="ExternalInput")
    gb_t = nc.dram_tensor("gb", [BLK, 4], F32, kind="ExternalInput")
    b2b_t = nc.dram_tensor("b2b", [BLK, n_out], F32, kind="ExternalInput")
    iota_t = nc.dram_tensor("iota", [BLK, BLK], F32, kind="ExternalInput")
    out_t = nc.dram_tensor("out", [nsh, n_out], F32, kind="ExternalOutput")

    h2b_t = nc.dram_tensor("h2b", [nsh, n_out], F32)  # allgather bounce in
    h2full_t = nc.dram_tensor("h2full", [npad, n_out], F32, addr_space="Shared")
    stat_in_t = nc.dram_tensor("statin", [BLK, 4], F32)
    stat_out_t = nc.dram_tensor("statout", [BLK, 4], F32, addr_space="Shared")
    if dbg:
        dbg_statw_t = nc.dram_tensor("dbg_statw", [BLK, 4], F32, kind="ExternalOutput")
        dbg_statr_t = nc.dram_tensor("dbg_statr", [BLK, 4], F32, kind="ExternalOutput")
        dbg_agg_t = nc.dram_tensor("dbg_agg", [BLK, BLK], F32, kind="ExternalOutput")
        dbg_h1_t = nc.dram_tensor("dbg_h1", [BLK, BLK], F32, kind="ExternalOutput")
        dbg_h2b_t = nc.dram_tensor("dbg_h2b", [nsh, n_out], F32, kind="ExternalOutput")
        dbg_h2f_t = nc.dram_tensor("dbg_h2f", [BLK, n_out], F32, kind="ExternalOutput")
        dbg_coef_t = nc.dram_tensor("dbg_coef", [BLK, 4], F32, kind="ExternalOutput")
        dbg_gt0_t = nc.dram_tensor("dbg_gt0", [BLK, n_in], F32, kind="ExternalOutput")
        dbg_sc0_t = nc.dram_tensor("dbg_sc0", [BLK, n_in], F32, kind="ExternalOutput")
        dbg_sel0_t = nc.dram_tensor("dbg_sel0", [BLK, BLK], F32, kind="ExternalOutput")

    groups = [list(range(CORES))]

    with tile.TileContext(nc) as tc:
        with (
            tc.tile_pool(name="persist", bufs=1) as pp,
            tc.tile_pool(name="work", bufs=4) as wp,
            tc.tile_pool(name="small", bufs=2) as sp,
        ):
            # --- resident data ---
            ec = pp.tile([BLK, ncols], I32, tag="ec")
            ljc = pp.tile([BLK, ncols], F32, tag="ljc")
            nmc = pp.tile([BLK, ncols], F32, tag="nmc")
            nc.sync.dma_start(ec[:], idx_t.ap())
            nc.sync.dma_start(ljc[:], lj_t.ap())
            nc.sync.dma_start(nmc[:], nm_t.ap())

            iota = pp.tile([BLK, BLK], F32, tag="iota")
            nc.sync.dma_start(iota[:], iota_t.ap())
            ident = pp.tile([BLK, BLK], F32, tag="ident")
            make_identity(nc, ident[:])

            w1c = []
            for c in range(2):
                w = pp.tile([BLK, BLK], F32, tag=f"w1c{c}")
                nc.sync.dma_start(w[:], w1_t.ap()[:, c * BLK:(c + 1) * BLK])
                w1c.append(w)
            w2c = []
            for c in range(2):
                w = pp.tile([BLK, n_out], F32, tag=f"w2c{c}")
                nc.sync.dma_start(w[:], w2_t.ap()[c * BLK:(c + 1) * BLK, :])
                w2c.append(w)
            gb = pp.tile([BLK, 4], F32, tag="gb")
            nc.sync.dma_start(gb[:], gb_t.ap())
            b2b = pp.tile([BLK, n_out], F32, tag="b2b")
            nc.sync.dma_start(b2b[:], b2b_t.ap())

            h1T = [pp.tile([BLK, nsh], F32, tag=f"h1T{c}", name=f"h1T{c}")
                   for c in range(2)]

            ssum = [pp.tile([BLK, 1], F32, tag=f"ssum{c}", name=f"ssum{c}")
                    for c in range(2)]
            sqsum = [pp.tile([BLK, 1], F32, tag=f"sqsum{c}", name=f"sqsum{c}")
                     for c in range(2)]
            for c in range(2):
                nc.vector.memset(ssum[c][:], 0.0)
                nc.vector.memset(sqsum[c][:], 0.0)

            # ---------------- Phase A: layer-1 aggregation + mm1 + stats ----
            with tc.tile_pool(name="psA", bufs=2, space="PSUM") as psA, \
                 tc.tile_pool(name="psH", bufs=4, space="PSUM") as psH:
                for b in range(nb_pc):
                    aggp = psA.tile([BLK, BLK], F32, tag="aggp")
                    selbs = []
                    if eq_batch > 1:
                        for q in range(tpb // eq_batch):
                            selb = wp.tile([BLK, eq_batch * BLK], F32,
                                           tag="selb", name=f"selbA{b}_{q}")
                            q0 = b * tpb + q * eq_batch
                            nc.vector.tensor_tensor(
                                out=selb[:].rearrange("p (g n) -> p g n",
                                                      g=eq_batch),
                                in0=ljc[:, q0:q0 + eq_batch]
                                    .rearrange("p (g o) -> p g o", o=1)
                                    .to_broadcast([BLK, eq_batch, BLK]),
                                in1=iota[:].rearrange("p (o n) -> p o n", o=1)
                                    .to_broadcast([BLK, eq_batch, BLK]),
                                op=ALU.is_equal)
                            selbs.append(selb)
                    for g in range(tpb // gather_g):
                        gt = wp.tile([BLK, gather_g * n_in], F32, tag="gt")
                        c0 = b * tpb + g * gather_g
                        gdst = (gt[:] if gather_g == 1 else
                                gt[:].rearrange("p (g d) -> p g d", g=gather_g))
                        nc.gpsimd.indirect_dma_start(
                            out=gdst,
                            out_offset=None,
                            in_=x_t.ap(),
                            in_offset=bass.IndirectOffsetOnAxis(
                                ap=ec[:, c0:c0 + gather_g], axis=0),
                        )
                        for j in range(gather_g):
                            t = g * gather_g + j
                            col = b * tpb + t
                            sc = wp.tile([BLK, n_in], F32, tag="sc")
                            nc.scalar.activation(
                                sc[:], gt[:, j * n_in:(j + 1) * n_in], AF.Copy,
                                scale=nmc[:, col:col + 1])
                            sel = wp.tile([BLK, BLK], F32, tag="sel")
                            nc.vector.tensor_tensor(
                                out=sel[:],
                                in0=ljc[:, col:col + 1].to_broadcast([BLK, BLK]),
                                in1=iota[:],
                                op=ALU.is_equal)
                            if dbg and b == 0 and t == 1:
                                nc.sync.dma_start(dbg_gt0_t.ap(),
                                                  gt[:, j * n_in:(j + 1) * n_in])
                                nc.sync.dma_start(dbg_sc0_t.ap(), sc[:])
                                nc.sync.dma_start(dbg_sel0_t.ap(), sel[:])
                            nc.tensor.matmul(
                                aggp[:], lhsT=sc[:], rhs=sel[:],
                                start=(t == 0), stop=(t == tpb - 1))
                    aggs = wp.tile([BLK, BLK], F32, tag="aggs")
                    nc.vector.tensor_copy(aggs[:], aggp[:])
                    if dbg and b == 0:
                        nc.sync.dma_start(dbg_agg_t.ap(), aggs[:])
                    for c in range(2):
                        h1p = psH.tile([BLK, BLK], F32, tag="h1p")
                        nc.tensor.matmul(h1p[:], lhsT=w1c[c][:], rhs=aggs[:],
                                         start=True, stop=True)
                        nc.vector.tensor_copy(
                            h1T[c][:, b * BLK:(b + 1) * BLK], h1p[:])
                        if dbg and b == 0 and c == 0:
                            nc.sync.dma_start(dbg_h1_t.ap(),
                                              h1T[c][:, :BLK])
                        red = sp.tile([BLK, 1], F32, tag="red")
                        nc.vector.reduce_sum(red[:], h1p[:],
                                             axis=mybir.AxisListType.X)
                        nc.vector.tensor_add(ssum[c][:], ssum[c][:], red[:])
                        sq = wp.tile([BLK, BLK], F32, tag="sq")
                        red2 = sp.tile([BLK, 1], F32, tag="red2")
                        nc.scalar.activation(sq[:], h1p[:], AF.Square,
                                             accum_out=red2[:])
                        nc.vector.tensor_add(sqsum[c][:], sqsum[c][:], red2[:])

            # ---------------- BN stats AllReduce + affine coefficients ------
            statw = sp.tile([BLK, 4], F32, tag="statw")
            for c in range(2):
                nc.vector.tensor_copy(statw[:, c:c + 1], ssum[c][:])
                nc.vector.tensor_copy(statw[:, 2 + c:3 + c], sqsum[c][:])
            nc.sync.dma_start(stat_in_t.ap(), statw[:])
            if dbg:
                nc.sync.dma_start(dbg_statw_t.ap(), statw[:])
            nc.gpsimd.collective_compute(
                "AllReduce", ALU.add, replica_groups=groups,
                ins=[stat_in_t.ap()], outs=[stat_out_t.ap()])
            statr = sp.tile([BLK, 4], F32, tag="statr")
            nc.sync.dma_start(statr[:], stat_out_t.ap())
            if dbg:
                nc.sync.dma_start(dbg_statr_t.ap(), statr[:])

            scale_c, shift_c = [], []
            for c in range(2):
                mu = sp.tile([BLK, 1], F32, tag=f"mu{c}")
                nc.vector.tensor_scalar_mul(mu[:], statr[:, c:c + 1], inv_n)
                ex2 = sp.tile([BLK, 1], F32, tag=f"ex2{c}")
                nc.vector.tensor_scalar_mul(ex2[:], statr[:, 2 + c:3 + c], inv_n)
                mu2 = sp.tile([BLK, 1], F32, tag=f"mu2{c}")
                nc.vector.tensor_mul(mu2[:], mu[:], mu[:])
                var = sp.tile([BLK, 1], F32, tag=f"var{c}")
                nc.vector.tensor_sub(var[:], ex2[:], mu2[:])
                nc.vector.tensor_scalar_add(var[:], var[:], EPS)
                sd = sp.tile([BLK, 1], F32, tag=f"sd{c}")
                nc.scalar.activation(sd[:], var[:], AF.Sqrt)
                inv = sp.tile([BLK, 1], F32, tag=f"inv{c}")
                nc.vector.reciprocal(inv[:], sd[:])
                sc_ = sp.tile([BLK, 1], F32, tag=f"scc{c}")
                nc.vector.tensor_mul(sc_[:], gb[:, c:c + 1], inv[:])
                sh_ = sp.tile([BLK, 1], F32, tag=f"shc{c}")
                nc.vector.tensor_mul(sh_[:], mu[:], sc_[:])
                nc.vector.tensor_sub(sh_[:], gb[:, 2 + c:3 + c], sh_[:])
                scale_c.append(sc_)
                shift_c.append(sh_)

            # ---------------- Phase B: BN+relu, mm2, transpose, write h2b ---
            with tc.tile_pool(name="psB", bufs=2, space="PSUM") as psB, \
                 tc.tile_pool(name="psT", bufs=2, space="PSUM") as psT:
                for b in range(nb_pc):
                    h2p = psB.tile([n_out, BLK], F32, tag="h2p")
                    for c in range(2):
                        y = wp.tile([BLK, BLK], F32, tag="y")
                        nc.scalar.activation(
                            y[:], h1T[c][:, b * BLK:(b + 1) * BLK], AF.Relu,
                            bias=shift_c[c][:, :1], scale=scale_c[c][:, :1])
                        nc.tensor.matmul(h2p[:], lhsT=w2c[c][:], rhs=y[:],
                                         start=(c == 0), stop=(c == 1))
                    h2s = wp.tile([n_out, BLK], F32, tag="h2s")
                    nc.vector.tensor_copy(h2s[:], h2p[:])
                    h2tp = psT.tile([BLK, n_out], F32, tag="h2tp")
                    nc.tensor.matmul(h2tp[:], lhsT=h2s[:],
                                     rhs=ident[:n_out, :n_out],
                                     start=True, stop=True)
                    h2ts = wp.tile([BLK, n_out], F32, tag="h2ts")
                    nc.vector.tensor_copy(h2ts[:], h2tp[:])
                    nc.sync.dma_start(
                        h2b_t.ap()[b * BLK:(b + 1) * BLK, :], h2ts[:])

            if dbg:
                coefw = sp.tile([BLK, 4], F32, tag="coefw")
                for c in range(2):
                    nc.vector.tensor_copy(coefw[:, c:c + 1], scale_c[c][:])
                    nc.vector.tensor_copy(coefw[:, 2 + c:3 + c], shift_c[c][:])
                nc.sync.dma_start(dbg_coef_t.ap(), coefw[:])
                dbh = wp.tile([nsh if nsh <= 256 else BLK, n_out], F32, tag="dbh")
                nc.sync.dma_start(dbh[:], h2b_t.ap()[:dbh.shape[0], :])
                nc.sync.dma_start(dbg_h2b_t.ap()[:dbh.shape[0], :], dbh[:])

            # ---------------- AllGather layer-2 input -----------------------
            nc.gpsimd.collective_compute(
                "AllGather", ALU.bypass, replica_groups=groups,
                ins=[h2b_t.ap()], outs=[h2full_t.ap()])

            if dbg:
                dbf = wp.tile([BLK, n_out], F32, tag="dbf")
                nc.sync.dma_start(dbf[:], h2full_t.ap()[3 * BLK:4 * BLK, :])
                nc.sync.dma_start(dbg_h2f_t.ap(), dbf[:])

            # ---------------- Phase C: layer-2 aggregation + bias + out -----
            g2 = gather_g2 or (gather_g * 2 if tpb % (gather_g * 2) == 0 else gather_g)
            with tc.tile_pool(name="psC", bufs=2, space="PSUM") as psC:
                for b in range(nb_pc):
                    outp = psC.tile([BLK, n_out], F32, tag="outp")
                    selbs = []
                    if eq_batch > 1:
                        for q in range(tpb // eq_batch):
                            selb = wp.tile([BLK, eq_batch * BLK], F32,
                                           tag="selb", name=f"selbC{b}_{q}")
                            q0 = b * tpb + q * eq_batch
                            nc.vector.tensor_tensor(
                                out=selb[:].rearrange("p (g n) -> p g n",
                                                      g=eq_batch),
                                in0=ljc[:, q0:q0 + eq_batch]
                                    .rearrange("p (g o) -> p g o", o=1)
                                    .to_broadcast([BLK, eq_batch, BLK]),
                                in1=iota[:].rearrange("p (o n) -> p o n", o=1)
                                    .to_broadcast([BLK, eq_batch, BLK]),
                                op=ALU.is_equal)
                            selbs.append(selb)
                    for g in range(tpb // g2):
                        gt = wp.tile([BLK, g2 * n_out], F32, tag="gt2")
                        c0 = b * tpb + g * g2
                        gdst = (gt[:] if g2 == 1 else
                                gt[:].rearrange("p (g d) -> p g d", g=g2))
                        nc.gpsimd.indirect_dma_start(
                            out=gdst,
                            out_offset=None,
                            in_=h2full_t.ap(),
                            in_offset=bass.IndirectOffsetOnAxis(
                                ap=ec[:, c0:c0 + g2], axis=0),
                        )
                        for j in range(g2):
                            t = g * g2 + j
                            col = b * tpb + t
                            sc = wp.tile([BLK, n_out], F32, tag="sc2")
                            nc.scalar.activation(
                                sc[:], gt[:, j * n_out:(j + 1) * n_out],
                                AF.Copy, scale=nmc[:, col:col + 1])
                            if eq_batch > 1:
                                sel_ap = selbs[t // eq_batch][
                                    :, (t % eq_batch) * BLK:
                                       (t % eq_batch + 1) * BLK]
                            else:
                                sel = wp.tile([BLK, BLK], F32, tag="sel2")
                                nc.vector.tensor_tensor(
                                    out=sel[:],
                                    in0=ljc[:, col:col + 1]
                                        .to_broadcast([BLK, BLK]),
                                    in1=iota[:],
                                    op=ALU.is_equal)
                                sel_ap = sel[:]
                            nc.tensor.matmul(
                                outp[:], lhsT=sel_ap, rhs=sc[:],
                                start=(t == 0), stop=(t == tpb - 1))
                    outs = wp.tile([BLK, n_out], F32, tag="outs")
                    nc.vector.tensor_add(outs[:], outp[:], b2b[:])
                    nc.sync.dma_start(
                        out_t.ap()[b * BLK:(b + 1) * BLK, :], outs[:])

    nc.compile()
    return nc


def _run(nc, inputs, n_nodes, nb_pc, tpb, n_in, n_hid, n_out, trace=False):
    x = np.ascontiguousarray(inputs["x"], dtype=np.float32)
    idx, lj, nm, _ = _host_prep(inputs["edge_index"], n_nodes, nb_pc)
    w1 = np.ascontiguousarray(inputs["W1"], dtype=np.float32)
    w2 = np.ascontiguousarray(inputs["W2"], dtype=np.float32)
    gamma = np.asarray(inputs["gamma1"], np.float32).reshape(2, BLK)
    beta = np.asarray(inputs["beta1"], np.float32).reshape(2, BLK)
    gb = np.stack([gamma[0], gamma[1], beta[0], beta[1]], axis=1).copy()
    b2b = np.tile(np.asarray(inputs["b2"], np.float32), (BLK, 1)).copy()
    iota = np.tile(np.arange(BLK, dtype=np.float32), (BLK, 1)).copy()

    ncols = nb_pc * tpb
    in_maps = []
    for k in range(CORES):
        sl = slice(k * ncols, (k + 1) * ncols)
        in_maps.append({
            "x": x, "eidx": np.ascontiguousarray(idx[:, sl]),
            "elj": np.ascontiguousarray(lj[:, sl]),
            "enm": np.ascontiguousarray(nm[:, sl]),
            "w1": w1, "w2": w2, "gb": gb, "b2b": b2b, "iota": iota,
        })
    res = bass_utils.run_bass_kernel_spmd(
        nc, in_maps, core_ids=list(range(CORES)), trace=trace)
    out = np.concatenate([r["out"] for r in res.results], axis=0)[:n_nodes]
    return out, res


# ------------------------- public entry point ---------------------------
_N, _IN, _HID, _OUT = 100000, 128, 256, 64
_NB_PC = 98  # blocks per core; NPAD = 8*98*128 = 100352


def kernel(**inputs) -> np.ndarray:
    idx, lj, nm, tpb = _host_prep(inputs["edge_index"], _N, _NB_PC)
    nc = build_kernel(_N, _IN, _HID, _OUT, _NB_PC, tpb)
    out, _ = _run(nc, inputs, _N, _NB_PC, tpb, _IN, _HID, _OUT)
    return out.astype(np.float32, copy=False)


# ------------------------- small self-test ------------------------------
def _np_reference(x, edge_index, W1, b1, gamma1, beta1, W2, b2):
    N = x.shape[0]
    src, dst = edge_index[0], edge_index[1]
    loop = np.arange(N)
    s = np.concatenate([src, loop])
    d = np.concatenate([dst, loop])
    deg = np.bincount(d, minlength=N).astype(np.float32)
    dis = 1.0 / np.sqrt(deg)
    norm = dis[s] * dis[d]

    def conv(h, W, b):
        hw = h @ W
        out = np.zeros((N, W.shape[1]), np.float32)
        np.add.at(out, d, hw[s] * norm[:, None])
        return out + b

    h = conv(x, W1, b1)
    mu, var = h.mean(0), h.var(0)
    h = np.maximum((h - mu) / np.sqrt(var + EPS) * gamma1 + beta1, 0)
    return conv(h, W2, b2)


def _selftest():
    rng = np.random.default_rng(0)
    n, e, nb_pc = 1024, 6000, 1
    x = rng.standard_normal((n, 128), dtype=np.float32)
    ei = rng.integers(0, n, (2, e)).astype(np.int64)
    w1 = rng.standard_normal((128, 256), dtype=np.float32) / 16
    w2 = rng.standard_normal((256, 64), dtype=np.float32) / 16
    g1 = rng.standard_normal(256).astype(np.float32)
    be1 = rng.standard_normal(256).astype(np.float32)
    b2 = rng.standard_normal(64).astype(np.float32)
    inputs = dict(x=x, edge_index=ei, W1=w1, b1=np.zeros(256, np.float32),
                  gamma1=g1, beta1=be1, W2=w2, b2=b2)
    idx, lj, nm, tpb = _host_prep(ei, n, nb_pc)
    print(f"selftest: tpb={tpb}")
    import os
    gg = int(os.environ.get('GG', '4')); gg2 = int(os.environ.get('GG2', '0')) or None
    eqb = int(os.environ.get('EQB', '4'))
    nc = build_kernel(n, 128, 256, 64, nb_pc, tpb, gather_g=gg, gather_g2=gg2,
                      dbg=True, eq_batch=eqb)
    out, res = _run(nc, inputs, n, nb_pc, tpb, 128, 256, 64)
    # stage-by-stage comparison vs numpy emulation
    iota = np.arange(BLK, dtype=np.float32)
    nblocks = nb_pc * CORES
    aggT = np.zeros((128, nblocks * 128), np.float32)
    for b in range(nblocks):
        for t in range(tpb):
            co = b * tpb + t
            sel = (lj[:, co:co + 1] == iota[None, :]).astype(np.float32)
            aggT[:, b * 128:(b + 1) * 128] += (
                x[idx[:, co]] * nm[:, co:co + 1]).T @ sel
    H1T = [(w1[:, :128]).T @ aggT, (w1[:, 128:]).T @ aggT]
    e_sum = np.stack([H1T[0].sum(1), H1T[1].sum(1)], 1)
    e_sq = np.stack([(H1T[0] ** 2).sum(1), (H1T[1] ** 2).sum(1)], 1)
    mu = np.concatenate([H1T[0].sum(1), H1T[1].sum(1)]) / n
    ex2 = np.concatenate([(H1T[0] ** 2).sum(1), (H1T[1] ** 2).sum(1)]) / n
    inv = 1 / np.sqrt(ex2 - mu ** 2 + EPS)
    e_scale = g1 * inv
    e_shift = be1 - mu * e_scale
    H2 = np.zeros((nblocks * 128, 64), np.float32)
    for b in range(nblocks):
        h2p = np.zeros((64, 128), np.float32)
        for c in range(2):
            y = np.maximum(
                H1T[c][:, b * 128:(b + 1) * 128]
                * e_scale[c * 128:(c + 1) * 128, None]
                + e_shift[c * 128:(c + 1) * 128, None], 0)
            h2p += (w2[c * 128:(c + 1) * 128]).T @ y
        H2[b * 128:(b + 1) * 128] = h2p.T
    r0 = res.results[0]
    r3 = res.results[3]
    def cmp(name, got, exp):
        e = np.abs(got - exp).max()
        print(f"  {name}: absdiff {e:.3e}  (scale {np.abs(exp).max():.3e})")
    cmp("aggT blk0", r0["dbg_agg"], aggT[:, :128])
    cmp("h1 blk0 c0", r0["dbg_h1"], H1T[0][:, :128])
    core_sums = e_sum.reshape(128, 2)  # careful: per-core partials differ
    cmp("statw core0 sum(c0)", r0["dbg_statw"][:, 0],
        H1T[0][:, :128].sum(1))
    cmp("statr allreduce sum(c0)", r0["dbg_statr"][:, 0], H1T[0].sum(1))
    cmp("scale_c0", r0["dbg_coef"][:, 0], e_scale[:128])
    cmp("shift_c0", r0["dbg_coef"][:, 2], e_shift[:128])
    cmp("h2b core0", r0["dbg_h2b"], H2[:128])
    cmp("h2full[384:512] core0", r0["dbg_h2f"], H2[384:512])
    cmp("h2full[384:512] core3", r3["dbg_h2f"], H2[384:512])
    cmp("out", out, _np_reference(**inputs))
    co = 1
    cmp("gt0 (t=1)", r0["dbg_gt0"], x[idx[:, co]])
    cmp("sc0 (t=1)", r0["dbg_sc0"], x[idx[:, co]] * nm[:, co:co + 1])
    cmp("sel0 (t=1)", r0["dbg_sel0"],
        (lj[:, co:co + 1] == iota[None, :]).astype(np.float32))
    np.savez("/tmp/dbg2.npz", agg=r0["dbg_agg"], aggexp=aggT[:, :128],
             gt0=r0["dbg_gt0"], sc0=r0["dbg_sc0"], sel0=r0["dbg_sel0"],
             idx=idx, lj=lj, nm=nm, x=x)
    exp = _np_reference(**inputs)
    err = np.abs(out - exp).max() / (np.abs(exp).max() + 1e-9)
    print(f"selftest rel-err: {err:.3e}")
    assert err < 2e-3, err
    print("SELFTEST PASSED")


if __name__ == "__main__":
    _selftest()
